# revision 3
# baseline (speedup 1.0000x reference)
"""Trainium2 Bass kernel for the correlation-map embedding module (v3).

Math (per (b, nf) pair):
  f1d = bilinear_down28(feature_i[b, nf])                  # [C, 28, 28]
  f2sel[c, k] = bilinear sample of feature_j[b, nf] at the K knn grid points
  corr[k, :, :] = relu(sum_c f2sel[c, k] * f1d[c, :, :])   # [K, 28, 28]
  out[k] = corr[k] / sum_hw(exp(corr[k])) * 10

v3 structure (see v2 docstring for the math restructurings):
  - single lhsT f2sel[c,k] per pair (f2 taps pre-summed on DVE);
  - f1 tap adds folded into PSUM accumulation (4 matmuls / bank, same lhsT);
  - matmul operands in bf16 (DVE converts on write; PSUM stays f32);
  - DMA-count minimization to dodge Tile's 8-lane DMAHW round-robin, where
    a load 8 DMAs after a store waits for that store's (compute-gated)
    completion before issuing: host pre-transposes fi/fj to
    [NF, BPC, C, H, W] so each nf is ONE 3.2MB load; both pair outputs
    store as ONE 800KB DMA per nf; const rows ride a single DMA.
    11 DMAs total, so load lanes never alias store lanes;
  - a dependency-free dummy ap_gather right after init forces the GPSIMD
    ucode library load (~6us IRAM DMA) into the preamble shadow instead of
    blocking the first real gather;
  - per-nf program order: load fi, load fj, 8 tap-plane muls (DVE, only
    need fi), gather (needs fj), then per-pair gg chain + matmuls +
    ScalarE epilogue.

Sharding: pure data parallel - batch dim (16) split across 8 cores, 2 each.
"""

import numpy as np

# hardcoded problem shapes (grading calls kernel(**inputs) standalone)
B, NF, C, H, W = 16, 3, 128, 56, 56
G = 28
K = 128
NCORES = 8
BPC = B // NCORES  # 2
P = 128
QH = G * G // 2  # 392 psum columns per bank
NROW = 4 * G * G + P + NF * 4 * K  # merged const row: w4 | ones | gw

_CACHE = {}


def _axis_coords(n_in):
    # float32 arithmetic to match the jax reference bit-for-bit
    src = np.arange(G, dtype=np.float32) * np.float32((n_in - 1) / (G - 1))
    i0 = np.clip(np.floor(src).astype(np.int32), 0, n_in - 2)
    w = (src - i0.astype(np.float32)).astype(np.float32)
    return i0, w


def _host_consts(knn_inds):
    i0h, wh = _axis_coords(H)
    i0w, ww = _axis_coords(W)
    # the even/odd strided-AP downsample assumes taps are (2k, 2k+1)
    assert np.array_equal(i0h, 2 * np.arange(G)) and np.array_equal(i0w, 2 * np.arange(G))

    # fused 4-tap downsample product-weight planes, each [28*28]
    # tap order (u, t): u = H-axis tap, t = W-axis tap
    ah, bh = (1.0 - wh), wh
    aw, bw = (1.0 - ww), ww
    w4 = np.stack(
        [
            np.outer(ah, aw).reshape(-1),
            np.outer(ah, bw).reshape(-1),
            np.outer(bh, aw).reshape(-1),
            np.outer(bh, bw).reshape(-1),
        ]
    ).astype(np.float32)  # [4, 784]

    # gather indices/weights for the 4 bilinear taps of each knn point
    knn = np.asarray(knn_inds).astype(np.int64)  # [NF, K, 2]
    gidx2 = np.zeros((P, NF, 32), dtype=np.int16)
    gwts = np.zeros((NF, 4 * K), dtype=np.float32)
    for nf in range(NF):
        h2 = knn[nf, :, 1]
        w2 = knn[nf, :, 0]
        r0 = i0h[h2]
        c0 = i0w[w2]
        # d=2 gather: each index fetches the horizontally-contiguous tap pair
        # (r_u*W + c0, +1); index is in units of 2 elements (c0 even).
        # j = k*2 + u ordering: gathered tile is [P, K, 2, 2] = [P, K, 4]
        pos = np.stack(
            [(r0 * W + c0) // 2, ((r0 + 1) * W + c0) // 2], axis=1
        ).reshape(-1)  # [256]
        wt = np.stack(
            [ah[h2] * aw[w2], ah[h2] * bw[w2], bh[h2] * aw[w2], bh[h2] * bw[w2]],
            axis=1,
        ).reshape(-1)
        gwts[nf] = wt.astype(np.float32)
        # merged gather: one gather per nf covering both batches stacked in
        # one [P, 2*H*W] tile; j = b*256 + k*2 + u, b offset in d=2 units.
        # ap_gather index layout: gathered index j comes from partition j%16,
        # slot j//16 of its 16-partition group; replicate across the 8 groups
        pos2 = np.concatenate([pos, pos + H * W // 2])  # [512]
        wrapped2 = pos2.reshape(32, 16).T.astype(np.int16)  # [16, 32]
        gidx2[:, nf, :] = np.tile(wrapped2, (8, 1))
    # merged const row: w4 planes | ones(P) | gather weights
    row = np.concatenate(
        [w4.reshape(-1), np.ones(P, np.float32), gwts.reshape(-1)]
    ).astype(np.float32)[None, :]
    return row, gidx2


def _build_bass():
    import concourse.bacc as bacc
    import concourse.tile as tile
    from concourse import mybir

    f32 = mybir.dt.float32
    f32r = mybir.dt.float32r
    bf16 = mybir.dt.bfloat16
    i16 = mybir.dt.int16
    AF = mybir.ActivationFunctionType

    nc = bacc.Bacc()
    # host pre-transposed: [NF, BPC, C, H*W]
    fi = nc.dram_tensor("fi", [NF, BPC, C, H * W], f32, kind="ExternalInput")
    fj = nc.dram_tensor("fj", [NF, BPC, C, H * W], f32, kind="ExternalInput")
    row_d = nc.dram_tensor("crow", [1, NROW], f32r, kind="ExternalInput")
    gidx_d = nc.dram_tensor("gidx2", [P, NF * 32], i16, kind="ExternalInput")
    # output [NF, BPC, K, G*G]; host transposes back
    out_d = nc.dram_tensor("out", [NF, BPC, K, G * G], f32, kind="ExternalOutput")

    with tile.TileContext(nc) as tc:
        with (
            tc.tile_pool(name="consts", bufs=1) as consts,
            tc.tile_pool(name="feat2x", bufs=2) as feat2x,
            tc.tile_pool(name="feat1", bufs=2) as feat1,
            tc.tile_pool(name="work", bufs=2) as work,
            tc.tile_pool(name="psum", bufs=2, space="PSUM") as pspool,
            tc.tile_pool(name="bcpsum", bufs=2, space="PSUM") as bcpool,
            tc.tile_pool(name="outp", bufs=2) as outp,
        ):
            # dependency-free dummy gather: forces the GPSIMD gather-ucode
            # library load (~6us) into the preamble instead of the first
            # real gather's critical path
            zi = consts.tile([P, 8], i16, tag="zi")
            nc.vector.memset(zi, 0)
            zo = consts.tile([P, 32], i16, tag="zo")
            nc.gpsimd.ap_gather(
                zo, zi, zi[:, :1], channels=P, num_elems=4, d=2, num_idxs=16
            )

            # merged const row + gather indices, on the sync queue so they
            # complete before the big loads monopolize the fabric
            crow = consts.tile([1, NROW], f32r, tag="crow")
            nc.sync.dma_start(out=crow, in_=row_d[:, :])
            gidx_t = consts.tile([P, NF * 32], i16, tag="gidx")
            nc.sync.dma_start(out=gidx_t, in_=gidx_d[:, :])
            ones = crow[:, 4 * G * G : 4 * G * G + P]

            bc_tiles = []

            def pe_broadcast(row_ap, n):
                """[1, n] -> [P, n] via PE: out = ones.T @ row."""
                dst = consts.tile([P, n], f32, tag=f"bc{len(bc_tiles)}")
                done = 0
                while done < n:
                    chunk = min(512, n - done)
                    bps = bcpool.tile([P, 512], f32, tag="bps")
                    nc.tensor.matmul(
                        bps[:, :chunk],
                        lhsT=ones,
                        rhs=row_ap[:, done : done + chunk],
                        start=True,
                        stop=True,
                    )
                    nc.scalar.copy(dst[:, done : done + chunk], bps[:, :chunk])
                    done += chunk
                bc_tiles.append(dst)
                return dst

            w4_t = [
                pe_broadcast(crow[:, u * G * G : (u + 1) * G * G], G * G)
                for u in range(4)
            ]
            gw0 = 4 * G * G + P
            gw_t = [
                pe_broadcast(crow[:, gw0 + nf * 4 * K : gw0 + (nf + 1) * 4 * K], 4 * K)
                for nf in range(NF)
            ]

            for nf in range(NF):
                # one 3.2MB load per tensor per nf: fi first (feeds the
                # DVE tap muls), fj second (feeds the gather)
                f1x = feat1.tile([P, BPC, H * W], f32, tag="f1x")
                nc.sync.dma_start(
                    out=f1x, in_=fi[nf].rearrange("b p q -> p b q")
                )
                f2x = feat2x.tile([P, BPC, H * W], f32, tag="f2x")
                nc.sync.dma_start(
                    out=f2x, in_=fj[nf].rearrange("b p q -> p b q")
                )

                # 4 weighted tap planes per batch on the full 28x28 grid
                # (DVE); the tap summation rides the PSUM accumulation.
                # These only need f1x, so they run while fj streams in.
                m = {}
                for b in range(BPC):
                    f1v = f1x[:, b].rearrange(
                        "p (h uu w tt) -> p h uu w tt", h=G, uu=2, w=G, tt=2
                    )
                    for u in range(2):
                        for t in range(2):
                            mt = work.tile([P, G * G], bf16, tag=f"m{b}{u}{t}")
                            nc.vector.tensor_mul(
                                mt.rearrange("p (h w) -> p h w", w=G),
                                f1v[:, :, u, :, t],
                                w4_t[2 * u + t].rearrange("p (h w) -> p h w", w=G),
                            )
                            m[(b, 2 * u + t)] = mt

                # f2 at the K selected grid points: both batches' 4 bilinear
                # taps in one GPSIMD gather
                g2 = work.tile([P, BPC, K, 4], f32, tag="g2")
                nc.gpsimd.ap_gather(
                    g2.rearrange("p b k t -> p (b k t)"),
                    f2x.rearrange("p b q -> p (b q)"),
                    gidx_t[:, nf * 32 : (nf + 1) * 32],
                    channels=P,
                    num_elems=BPC * H * W // 2,
                    d=2,
                    num_idxs=BPC * 2 * K,
                )

                o2 = outp.tile([P, BPC, G * G], f32, tag="o2")
                for b in range(BPC):
                    # tap weights, then pre-sum the 4 taps -> single lhsT
                    gg = work.tile([P, K, 4], f32, tag="gg")
                    nc.vector.tensor_mul(
                        gg.rearrange("p k t -> p (k t)"),
                        g2[:, b].rearrange("p k t -> p (k t)"),
                        gw_t[nf],
                    )
                    ggv = gg.rearrange("p k (x two) -> p (k x) two", two=2)
                    h1 = work.tile([P, 2 * K], f32, tag="h1")
                    nc.vector.tensor_add(h1, ggv[:, :, 0], ggv[:, :, 1])
                    h1v = h1.rearrange("p (k two) -> p k two", two=2)
                    f2sel = work.tile([P, K], bf16, tag="f2sel")
                    nc.vector.tensor_add(f2sel, h1v[:, :, 0], h1v[:, :, 1])

                    # corr[k, q] = sum_c f2sel[c,k] * sum_u m_u[c,q]
                    ps = pspool.tile([P, 2, 512], f32, tag="ps")
                    for half in range(2):
                        lo = half * QH
                        for u4 in range(4):
                            nc.tensor.matmul(
                                ps[:, half, :QH],
                                lhsT=f2sel,
                                rhs=m[(b, u4)][:, lo : lo + QH],
                                start=(u4 == 0),
                                stop=(u4 == 3),
                            )

                    # epilogue on ScalarE: r = 10*relu(corr); s = sum(exp(r/10));
                    # out = r * (1/s)
                    r = outp.tile([P, 2, QH], f32, tag="r")
                    nc.scalar.activation(r, ps[:, :, :QH], AF.Relu, scale=10.0)
                    rf = r.rearrange("p h q -> p (h q)")  # [P, 784] contiguous
                    e = work.tile([P, G * G], bf16, tag="e")
                    s = work.tile([P, 1], f32, tag="s")
                    nc.scalar.activation(e, rf, AF.Exp, scale=0.1, accum_out=s)
                    rec = work.tile([P, 1], f32, tag="rec")
                    nc.vector.reciprocal(rec, s)
                    nc.scalar.mul(o2[:, b], rf, rec)

                # one 800KB store per nf covering both batches, issued from
                # ScalarE right after the second batch's normalize
                nc.scalar.dma_start(
                    out=out_d[nf].rearrange("b p q -> p b q"), in_=o2
                )
    return nc


def _get_bass():
    if "nc" not in _CACHE:
        nc = _build_bass()
        if not nc.is_finalized():
            nc.finalize()
        _CACHE["nc"] = nc
    return _CACHE["nc"]


def _prepare_in_maps(feature_i, feature_j, knn_inds):
    row, gidx2 = _host_consts(knn_inds)
    fi = np.asarray(feature_i, dtype=np.float32).reshape(NCORES, BPC, NF, C, H * W)
    fj = np.asarray(feature_j, dtype=np.float32).reshape(NCORES, BPC, NF, C, H * W)
    # [core, b, nf, c, q] -> [core, nf, b, c, q]
    fi = np.ascontiguousarray(fi.transpose(0, 2, 1, 3, 4))
    fj = np.ascontiguousarray(fj.transpose(0, 2, 1, 3, 4))
    in_maps = []
    for core in range(NCORES):
        in_maps.append(
            {
                "fi": fi[core],
                "fj": fj[core],
                "crow": row,
                "gidx2": gidx2.reshape(P, NF * 32),
            }
        )
    return in_maps


def kernel(feature_i, feature_j, mask, optical_flow, knn_inds):
    from concourse import bass_utils

    nc = _get_bass()
    in_maps = _prepare_in_maps(feature_i, feature_j, knn_inds)

    res = bass_utils.run_bass_kernel_spmd(nc, in_maps, core_ids=list(range(NCORES)))
    # [NF, BPC, K, G*G] per core -> [B, NF, K, G, G]
    out = np.stack([res.results[c]["out"] for c in range(NCORES)], axis=0)
    out = out.reshape(NCORES, NF, BPC, K, G, G).transpose(0, 2, 1, 3, 4, 5)
    return np.ascontiguousarray(out.reshape(B, NF, K, G, G)).astype(np.float32)


# revision 5
# speedup vs baseline: 1.4984x; 1.4984x over previous
"""Trainium2 Bass kernel for the correlation-map embedding module (v4).

Math (per (b, nf) pair):
  f1d = bilinear_down28(feature_i[b, nf])                  # [C, 28, 28]
  f2sel[c, k] = bilinear sample of feature_j[b, nf] at the K knn grid points
  corr[k, :, :] = relu(sum_c f2sel[c, k] * f1d[c, :, :])   # [K, 28, 28]
  out[k] = corr[k] / sum_hw(exp(corr[k])) * 10

v4 key changes over v3 (which was paced at ~21us/nf by ap_gather - the
GPSIMD software gather takes ~15-21us of invisible Q7 time per call):
  - the f2 tap fetch is a hardware SWDGE dma_gather(transpose=True)
    STRAIGHT FROM HBM: the host pre-packs feature_j as [spatial, channel]
    f16 rows (256B each), the gather pulls only the 1024 tap rows per nf
    (256KB instead of the full 3.2MB fj load) and the XBAR transpose
    lands them channel-on-partition. fj HBM traffic drops 12x and the
    Q7 gather disappears;
  - feature_i is host-cast to f16: halves fi traffic and doubles the
    DVE tap-mul rate (16-bit 2x mode);
  - all loads + gathers are issued up-front (pools sized to hold all 3
    nf), so the per-nf compute only waits on its own data.
Per-core HBM traffic: fi 4.8MB + fj-gather 0.77MB + out 2.4MB ~= 8MB.

Sharding: pure data parallel - batch dim (16) split across 8 cores, 2 each.
"""

import numpy as np

# hardcoded problem shapes (grading calls kernel(**inputs) standalone)
B, NF, C, H, W = 16, 3, 128, 56, 56
G = 28
K = 128
NCORES = 8
BPC = B // NCORES  # 2
P = 128
QH = G * G // 2  # 392 psum columns per bank
NIDX = BPC * K * 4  # 1024 gather rows per nf
NROW = 4 * G * G + P + NF * 4 * K  # merged const row: w4 | ones | gw

_CACHE = {}


def _axis_coords(n_in):
    # float32 arithmetic to match the jax reference bit-for-bit
    src = np.arange(G, dtype=np.float32) * np.float32((n_in - 1) / (G - 1))
    i0 = np.clip(np.floor(src).astype(np.int32), 0, n_in - 2)
    w = (src - i0.astype(np.float32)).astype(np.float32)
    return i0, w


def _host_consts(knn_inds):
    i0h, wh = _axis_coords(H)
    i0w, ww = _axis_coords(W)
    assert np.array_equal(i0h, 2 * np.arange(G)) and np.array_equal(i0w, 2 * np.arange(G))

    # fused 4-tap downsample product-weight planes, each [28*28]
    ah, bh = (1.0 - wh), wh
    aw, bw = (1.0 - ww), ww
    w4 = np.stack(
        [
            np.outer(ah, aw).reshape(-1),
            np.outer(ah, bw).reshape(-1),
            np.outer(bh, aw).reshape(-1),
            np.outer(bh, bw).reshape(-1),
        ]
    ).astype(np.float32)  # [4, 784]

    knn = np.asarray(knn_inds).astype(np.int64)  # [NF, K, 2]
    gidx = np.zeros((P, NF, NIDX // 16), dtype=np.int16)
    gwts = np.zeros((NF, 4 * K), dtype=np.float32)
    for nf in range(NF):
        h2 = knn[nf, :, 1]
        w2 = knn[nf, :, 0]
        r0 = i0h[h2]
        c0 = i0w[w2]
        # 4 tap rows per point, (u, t) order matching the weight order
        taps = np.stack(
            [r0 * W + c0, r0 * W + c0 + 1, (r0 + 1) * W + c0, (r0 + 1) * W + c0 + 1],
            axis=1,
        ).reshape(-1)  # [512]
        wt = np.stack(
            [ah[h2] * aw[w2], ah[h2] * bw[w2], bh[h2] * aw[w2], bh[h2] * bw[w2]],
            axis=1,
        ).reshape(-1)
        gwts[nf] = wt.astype(np.float32)
        # j = b*512 + k*4 + t over the [BPC*H*W, C] row space of this nf
        idx = np.concatenate([taps, taps + H * W]).astype(np.int16)  # [1024]
        # dma_gather index wrap: idx j lives at [j % 16, j // 16]
        wrapped = idx.reshape(NIDX // 16, 16).T  # [16, 64]
        gidx[:, nf, :] = np.tile(wrapped, (8, 1))
    row = np.concatenate(
        [w4.reshape(-1), np.ones(P, np.float32), gwts.reshape(-1)]
    ).astype(np.float32)[None, :]
    return row, gidx


def _build_bass():
    import concourse.bacc as bacc
    import concourse.tile as tile
    from concourse import mybir

    f32 = mybir.dt.float32
    f32r = mybir.dt.float32r
    f16 = mybir.dt.float16
    i16 = mybir.dt.int16
    AF = mybir.ActivationFunctionType

    nc = bacc.Bacc()
    # host pre-cast f16: [NF, BPC, C, H*W]
    fi = nc.dram_tensor("fi", [NF, BPC, C, H * W], f16, kind="ExternalInput")
    # host pre-packed gather source: rows of 128 channels per spatial pos
    fjt = nc.dram_tensor("fjt", [NF, BPC * H * W, C], f16, kind="ExternalInput")
    row_d = nc.dram_tensor("crow", [1, NROW], f32r, kind="ExternalInput")
    gidx_d = nc.dram_tensor("gidx", [P, NF * (NIDX // 16)], i16, kind="ExternalInput")
    out_d = nc.dram_tensor("out", [NF, BPC, K, G * G], f32, kind="ExternalOutput")

    with tile.TileContext(nc) as tc:
        with (
            tc.tile_pool(name="consts", bufs=1) as consts,
            tc.tile_pool(name="feat1", bufs=1) as feat1,
            tc.tile_pool(name="gat", bufs=1) as gat,
            tc.tile_pool(name="work", bufs=2) as work,
            tc.tile_pool(name="psum", bufs=2, space="PSUM") as pspool,
            tc.tile_pool(name="bcpsum", bufs=2, space="PSUM") as bcpool,
            tc.tile_pool(name="outp", bufs=3) as outp,
        ):
            # consts first (tiny, sync queue)
            crow = consts.tile([1, NROW], f32r, tag="crow")
            nc.sync.dma_start(out=crow, in_=row_d[:, :])
            gidx_t = consts.tile([P, NF * (NIDX // 16)], i16, tag="gidx")
            nc.sync.dma_start(out=gidx_t, in_=gidx_d[:, :])
            ones = crow[:, 4 * G * G : 4 * G * G + P]

            # dummy 128-idx gather forces the SWDGE ucode library load into
            # the preamble shadow; zero indices only need a memset
            zi = consts.tile([P, 8], i16, tag="zi")
            nc.vector.memset(zi, 0)
            zo = consts.tile([P, 1, 128], f16, tag="zo")
            nc.gpsimd.dma_gather(zo, fjt[0], zi, 128, 128, C, transpose=True)

            # all fi loads (sync queue) and tap-row gathers (SWDGE) up front
            f1xs = []
            for nf in range(NF):
                t = feat1.tile([P, BPC, H * W], f16, tag=f"f1x{nf}")
                nc.sync.dma_start(out=t, in_=fi[nf].rearrange("b p q -> p b q"))
                f1xs.append(t)
            g2s = []
            for nf in range(NF):
                g2 = gat.tile([P, 1, NIDX], f16, tag=f"g2{nf}")
                nc.gpsimd.dma_gather(
                    g2,
                    fjt[nf],
                    gidx_t[:, nf * (NIDX // 16) : (nf + 1) * (NIDX // 16)],
                    NIDX,
                    NIDX,
                    C,
                    transpose=True,
                    # >64 descriptors (1024 idx) overflows a single SWDGE
                    # packet and wedges the exec unit; let it split
                    single_packet=False,
                )
                g2s.append(g2)

            bc_tiles = []

            def pe_broadcast(row_ap, n, dtype):
                """[1, n] -> [P, n] via PE: out = ones.T @ row."""
                dst = consts.tile([P, n], dtype, tag=f"bc{len(bc_tiles)}")
                done = 0
                while done < n:
                    chunk = min(512, n - done)
                    bps = bcpool.tile([P, 512], f32, tag="bps")
                    nc.tensor.matmul(
                        bps[:, :chunk],
                        lhsT=ones,
                        rhs=row_ap[:, done : done + chunk],
                        start=True,
                        stop=True,
                    )
                    nc.scalar.copy(dst[:, done : done + chunk], bps[:, :chunk])
                    done += chunk
                bc_tiles.append(dst)
                return dst

            w4_t = [
                pe_broadcast(crow[:, u * G * G : (u + 1) * G * G], G * G, f16)
                for u in range(4)
            ]
            gw0 = 4 * G * G + P
            gw_t = [
                pe_broadcast(
                    crow[:, gw0 + nf * 4 * K : gw0 + (nf + 1) * 4 * K], 4 * K, f16
                )
                for nf in range(NF)
            ]

            for nf in range(NF):
                # 4 weighted tap planes per batch (DVE f16 2x)
                m = {}
                for b in range(BPC):
                    f1v = f1xs[nf][:, b].rearrange(
                        "p (h uu w tt) -> p h uu w tt", h=G, uu=2, w=G, tt=2
                    )
                    for u in range(2):
                        for t in range(2):
                            mt = work.tile([P, G * G], f16, tag=f"m{b}{u}{t}")
                            nc.vector.tensor_mul(
                                mt.rearrange("p (h w) -> p h w", w=G),
                                f1v[:, :, u, :, t],
                                w4_t[2 * u + t].rearrange("p (h w) -> p h w", w=G),
                            )
                            m[(b, 2 * u + t)] = mt

                gv = g2s[nf].rearrange("p one (b j) -> p (one b) j", b=BPC)
                o2 = outp.tile([P, BPC, G * G], f32, tag="o2")
                for b in range(BPC):
                    # tap weights, then pre-sum the 4 taps -> single lhsT
                    gg = work.tile([P, 4 * K], f16, tag="gg")
                    nc.vector.tensor_mul(gg, gv[:, b], gw_t[nf])
                    ggv = gg.rearrange("p (x two) -> p x two", two=2)
                    h1 = work.tile([P, 2 * K], f16, tag="h1")
                    nc.vector.tensor_add(h1, ggv[:, :, 0], ggv[:, :, 1])
                    h1v = h1.rearrange("p (k two) -> p k two", two=2)
                    f2sel = work.tile([P, K], f16, tag="f2sel")
                    nc.vector.tensor_add(f2sel, h1v[:, :, 0], h1v[:, :, 1])

                    # corr[k, q] = sum_c f2sel[c,k] * sum_u m_u[c,q]
                    ps = pspool.tile([P, 2, 512], f32, tag="ps")
                    for half in range(2):
                        lo = half * QH
                        for u4 in range(4):
                            nc.tensor.matmul(
                                ps[:, half, :QH],
                                lhsT=f2sel,
                                rhs=m[(b, u4)][:, lo : lo + QH],
                                start=(u4 == 0),
                                stop=(u4 == 3),
                            )

                    # epilogue on ScalarE: r = 10*relu(corr); s = sum(exp(r/10));
                    # out = r * (1/s)
                    r = outp.tile([P, 2, QH], f32, tag="r")
                    nc.scalar.activation(r, ps[:, :, :QH], AF.Relu, scale=10.0)
                    rf = r.rearrange("p h q -> p (h q)")
                    e = work.tile([P, G * G], f32, tag="e")
                    s = work.tile([P, 1], f32, tag="s")
                    nc.scalar.activation(e, rf, AF.Exp, scale=0.1, accum_out=s)
                    rec = work.tile([P, 1], f32, tag="rec")
                    nc.vector.reciprocal(rec, s)
                    nc.scalar.mul(o2[:, b], rf, rec)

                # one 800KB store per nf covering both batches
                nc.scalar.dma_start(
                    out=out_d[nf].rearrange("b p q -> p b q"), in_=o2
                )
    return nc


def _get_bass():
    if "nc" not in _CACHE:
        nc = _build_bass()
        if not nc.is_finalized():
            nc.finalize()
        _CACHE["nc"] = nc
    return _CACHE["nc"]


def _prepare_in_maps(feature_i, feature_j, knn_inds):
    row, gidx = _host_consts(knn_inds)
    fi = np.asarray(feature_i, dtype=np.float32).reshape(NCORES, BPC, NF, C, H * W)
    # [core, b, nf, c, q] -> [core, nf, b, c, q], f16
    fi = np.ascontiguousarray(fi.transpose(0, 2, 1, 3, 4)).astype(np.float16)
    fj = np.asarray(feature_j, dtype=np.float32).reshape(NCORES, BPC, NF, C, H * W)
    # [core, b, nf, c, q] -> [core, nf, b, q, c] rows of channels, f16
    fjt = np.ascontiguousarray(fj.transpose(0, 2, 1, 4, 3)).astype(np.float16)
    gidx2 = gidx.reshape(P, NF * (NIDX // 16))
    in_maps = []
    for core in range(NCORES):
        in_maps.append(
            {
                "fi": fi[core],
                "fjt": fjt[core].reshape(NF, BPC * H * W, C),
                "crow": row,
                "gidx": gidx2,
            }
        )
    return in_maps


def kernel(feature_i, feature_j, mask, optical_flow, knn_inds):
    from concourse import bass_utils

    nc = _get_bass()
    in_maps = _prepare_in_maps(feature_i, feature_j, knn_inds)

    res = bass_utils.run_bass_kernel_spmd(nc, in_maps, core_ids=list(range(NCORES)))
    out = np.stack([res.results[c]["out"] for c in range(NCORES)], axis=0)
    out = out.reshape(NCORES, NF, BPC, K, G, G).transpose(0, 2, 1, 3, 4, 5)
    return np.ascontiguousarray(out.reshape(B, NF, K, G, G)).astype(np.float32)


# revision 6
# speedup vs baseline: 1.6656x; 1.1116x over previous
"""Trainium2 Bass kernel for the correlation-map embedding module (v5).

Math (per (b, nf) pair):
  f1d = bilinear_down28(feature_i[b, nf])                  # [C, 28, 28]
  f2sel[c, k] = bilinear sample of feature_j[b, nf] at the K knn grid points
  corr[k, :, :] = relu(sum_c f2sel[c, k] * f1d[c, :, :])   # [K, 28, 28]
  out[k] = corr[k] / sum_hw(exp(corr[k])) * 10

v4 key changes over v3 (which was paced at ~21us/nf by ap_gather - the
GPSIMD software gather takes ~15-21us of invisible Q7 time per call):
  - the f2 tap fetch is a hardware SWDGE dma_gather(transpose=True)
    STRAIGHT FROM HBM: the host pre-packs feature_j as [spatial, channel]
    f16 rows (256B each), the gather pulls only the 1024 tap rows per nf
    (256KB instead of the full 3.2MB fj load) and the XBAR transpose
    lands them channel-on-partition. fj HBM traffic drops 12x and the
    Q7 gather disappears;
  - feature_i is host-cast to f16: halves fi traffic and doubles the
    DVE tap-mul rate (16-bit 2x mode);
  - all loads + gathers are issued up-front (pools sized to hold all 3
    nf), so the per-nf compute only waits on its own data.
Per-core HBM traffic: fi 4.8MB + fj-gather 0.77MB + out 2.4MB ~= 8MB.

v5 refinements (v4 measured 68us; the 4 dma_gathers burned ~48us of
GPSIMD descriptor generation because single_packet=False emits one
descriptor per index):
  - both batches' channels are packed into ONE 512B gather row
    ([NF, H*W, BPC*C] f16), halving num_idxs to 512 so each nf is a
    single single-packet gather (~34 aggregated descriptors);
  - the per-batch tap weighting is ONE fully-contiguous DVE multiply
    (f16 2x mode) against a host-interleaved (h,u,w,t)-order weight
    plane; the matmul moving operand takes the strided tap views
    instead of the DVE.

Sharding: pure data parallel - batch dim (16) split across 8 cores, 2 each.
"""

import numpy as np

# hardcoded problem shapes (grading calls kernel(**inputs) standalone)
B, NF, C, H, W = 16, 3, 128, 56, 56
G = 28
K = 128
NCORES = 8
BPC = B // NCORES  # 2
P = 128
QH = G * G // 2  # 392 psum columns per bank
NIDX = K * 4  # 512 gather rows per nf (both batches per row)
NROW = 4 * G * G + P + NF * 4 * K  # merged const row: w4il | ones | gw

_CACHE = {}


def _axis_coords(n_in):
    # float32 arithmetic to match the jax reference bit-for-bit
    src = np.arange(G, dtype=np.float32) * np.float32((n_in - 1) / (G - 1))
    i0 = np.clip(np.floor(src).astype(np.int32), 0, n_in - 2)
    w = (src - i0.astype(np.float32)).astype(np.float32)
    return i0, w


def _host_consts(knn_inds):
    i0h, wh = _axis_coords(H)
    i0w, ww = _axis_coords(W)
    assert np.array_equal(i0h, 2 * np.arange(G)) and np.array_equal(i0w, 2 * np.arange(G))

    # fused 4-tap downsample product-weight planes, each [28*28]
    ah, bh = (1.0 - wh), wh
    aw, bw = (1.0 - ww), ww
    # interleaved (gh, u, gw, t) order matching f1's raw memory order
    wh2 = np.stack([ah, bh], axis=1).reshape(-1)  # [56] = (gh, u)
    ww2 = np.stack([aw, bw], axis=1).reshape(-1)  # [56] = (gw, t)
    w4il = np.outer(wh2, ww2).reshape(-1).astype(np.float32)  # [3136]

    knn = np.asarray(knn_inds).astype(np.int64)  # [NF, K, 2]
    gidx = np.zeros((P, NF, NIDX // 16), dtype=np.int16)
    gwts = np.zeros((NF, 4 * K), dtype=np.float32)
    for nf in range(NF):
        h2 = knn[nf, :, 1]
        w2 = knn[nf, :, 0]
        r0 = i0h[h2]
        c0 = i0w[w2]
        # 4 tap rows per point, (u, t) order matching the weight order
        taps = np.stack(
            [r0 * W + c0, r0 * W + c0 + 1, (r0 + 1) * W + c0, (r0 + 1) * W + c0 + 1],
            axis=1,
        ).reshape(-1)  # [512]
        wt = np.stack(
            [ah[h2] * aw[w2], ah[h2] * bw[w2], bh[h2] * aw[w2], bh[h2] * bw[w2]],
            axis=1,
        ).reshape(-1)
        gwts[nf] = wt.astype(np.float32)
        # j = k*4 + t over the [H*W, BPC*C] row space of this nf
        idx = taps.astype(np.int16)  # [512]
        # dma_gather index wrap: idx j lives at [j % 16, j // 16]
        wrapped = idx.reshape(NIDX // 16, 16).T  # [16, 32]
        gidx[:, nf, :] = np.tile(wrapped, (8, 1))
    row = np.concatenate(
        [w4il, np.ones(P, np.float32), gwts.reshape(-1)]
    ).astype(np.float32)[None, :]
    return row, gidx


def _build_bass():
    import concourse.bacc as bacc
    import concourse.tile as tile
    from concourse import mybir

    f32 = mybir.dt.float32
    f32r = mybir.dt.float32r
    f16 = mybir.dt.float16
    i16 = mybir.dt.int16
    AF = mybir.ActivationFunctionType

    nc = bacc.Bacc()
    # host pre-cast f16: [NF, BPC, C, H*W]
    fi = nc.dram_tensor("fi", [NF, BPC, C, H * W], f16, kind="ExternalInput")
    # host pre-packed gather source: rows of 128 channels per spatial pos
    fjt = nc.dram_tensor("fjt", [NF, H * W, BPC * C], f16, kind="ExternalInput")
    row_d = nc.dram_tensor("crow", [1, NROW], f32r, kind="ExternalInput")
    gidx_d = nc.dram_tensor("gidx", [P, NF * (NIDX // 16)], i16, kind="ExternalInput")
    out_d = nc.dram_tensor("out", [NF, BPC, K, G * G], f32, kind="ExternalOutput")

    with tile.TileContext(nc) as tc:
        with (
            tc.tile_pool(name="consts", bufs=1) as consts,
            tc.tile_pool(name="feat1", bufs=1) as feat1,
            tc.tile_pool(name="gat", bufs=1) as gat,
            tc.tile_pool(name="work", bufs=2) as work,
            tc.tile_pool(name="psum", bufs=2, space="PSUM") as pspool,
            tc.tile_pool(name="bcpsum", bufs=2, space="PSUM") as bcpool,
            tc.tile_pool(name="outp", bufs=3) as outp,
        ):
            # consts first (tiny, sync queue)
            crow = consts.tile([1, NROW], f32r, tag="crow")
            nc.sync.dma_start(out=crow, in_=row_d[:, :])
            gidx_t = consts.tile([P, NF * (NIDX // 16)], i16, tag="gidx")
            nc.sync.dma_start(out=gidx_t, in_=gidx_d[:, :])
            ones = crow[:, 4 * G * G : 4 * G * G + P]

            # dummy 128-idx gather forces the SWDGE ucode library load into
            # the preamble shadow; zero indices only need a memset
            zi = consts.tile([P, 8], i16, tag="zi")
            nc.vector.memset(zi, 0)
            zo = consts.tile([P, BPC, 128], f16, tag="zo")
            nc.gpsimd.dma_gather(zo, fjt[0], zi, 128, 128, BPC * C, transpose=True)

            # all fi loads (sync queue) and tap-row gathers (SWDGE) up front
            f1xs = []
            for nf in range(NF):
                t = feat1.tile([P, BPC, H * W], f16, tag=f"f1x{nf}")
                nc.sync.dma_start(out=t, in_=fi[nf].rearrange("b p q -> p b q"))
                f1xs.append(t)
            g2s = []
            for nf in range(NF):
                # one gather per nf: each 512B row carries both batches'
                # channels; 512 idx -> ~34 aggregated descriptors, fits a
                # single SWDGE packet (>64 descriptors wedges the exec unit)
                g2 = gat.tile([P, BPC, NIDX], f16, tag=f"g2{nf}")
                nc.gpsimd.dma_gather(
                    g2,
                    fjt[nf],
                    gidx_t[:, nf * (NIDX // 16) : (nf + 1) * (NIDX // 16)],
                    NIDX,
                    NIDX,
                    BPC * C,
                    transpose=True,
                )
                g2s.append(g2)

            bc_tiles = []

            def pe_broadcast(row_ap, n, dtype):
                """[1, n] -> [P, n] via PE: out = ones.T @ row."""
                dst = consts.tile([P, n], dtype, tag=f"bc{len(bc_tiles)}")
                done = 0
                while done < n:
                    chunk = min(512, n - done)
                    bps = bcpool.tile([P, 512], f32, tag="bps")
                    nc.tensor.matmul(
                        bps[:, :chunk],
                        lhsT=ones,
                        rhs=row_ap[:, done : done + chunk],
                        start=True,
                        stop=True,
                    )
                    nc.scalar.copy(dst[:, done : done + chunk], bps[:, :chunk])
                    done += chunk
                bc_tiles.append(dst)
                return dst

            w4il_t = pe_broadcast(crow[:, : 4 * G * G], 4 * G * G, f16)
            gw0 = 4 * G * G + P
            gw_t = [
                pe_broadcast(
                    crow[:, gw0 + nf * 4 * K : gw0 + (nf + 1) * 4 * K], 4 * K, f16
                )
                for nf in range(NF)
            ]

            for nf in range(NF):
                # weighted taps: one fully-contiguous f16 multiply per batch
                # (f1 raw memory order (h,u,w,t) matches the interleaved w4)
                m = {}
                for b in range(BPC):
                    ma = work.tile([P, H * W], f16, tag=f"ma{b}")
                    nc.vector.tensor_mul(ma, f1xs[nf][:, b], w4il_t)
                    m[b] = ma.rearrange(
                        "p (h uu w tt) -> p h uu w tt", h=G, uu=2, w=G, tt=2
                    )

                gv = g2s[nf]
                o2 = outp.tile([P, BPC, G * G], f32, tag="o2")
                for b in range(BPC):
                    # tap weights, then pre-sum the 4 taps -> single lhsT
                    gg = work.tile([P, 4 * K], f16, tag="gg")
                    nc.vector.tensor_mul(gg, gv[:, b], gw_t[nf])
                    ggv = gg.rearrange("p (x two) -> p x two", two=2)
                    h1 = work.tile([P, 2 * K], f16, tag="h1")
                    nc.vector.tensor_add(h1, ggv[:, :, 0], ggv[:, :, 1])
                    h1v = h1.rearrange("p (k two) -> p k two", two=2)
                    f2sel = work.tile([P, K], f16, tag="f2sel")
                    nc.vector.tensor_add(f2sel, h1v[:, :, 0], h1v[:, :, 1])

                    # corr[k, q] = sum_c f2sel[c,k] * sum_u m_u[c,q]
                    ps = pspool.tile([P, 2, 512], f32, tag="ps")
                    GH = G // 2
                    for half in range(2):
                        hs = half * GH
                        for u4 in range(4):
                            u, t = divmod(u4, 2)
                            nc.tensor.matmul(
                                ps[:, half, :QH],
                                lhsT=f2sel,
                                rhs=m[b][:, hs : hs + GH, u, :, t],
                                start=(u4 == 0),
                                stop=(u4 == 3),
                            )

                    # epilogue on ScalarE: r = 10*relu(corr); s = sum(exp(r/10));
                    # out = r * (1/s)
                    r = outp.tile([P, 2, QH], f32, tag="r")
                    nc.scalar.activation(r, ps[:, :, :QH], AF.Relu, scale=10.0)
                    rf = r.rearrange("p h q -> p (h q)")
                    e = work.tile([P, G * G], f32, tag="e")
                    s = work.tile([P, 1], f32, tag="s")
                    nc.scalar.activation(e, rf, AF.Exp, scale=0.1, accum_out=s)
                    rec = work.tile([P, 1], f32, tag="rec")
                    nc.vector.reciprocal(rec, s)
                    nc.scalar.mul(o2[:, b], rf, rec)

                # one 800KB store per nf covering both batches
                nc.scalar.dma_start(
                    out=out_d[nf].rearrange("b p q -> p b q"), in_=o2
                )
    return nc


def _get_bass():
    if "nc" not in _CACHE:
        nc = _build_bass()
        if not nc.is_finalized():
            nc.finalize()
        _CACHE["nc"] = nc
    return _CACHE["nc"]


def _prepare_in_maps(feature_i, feature_j, knn_inds):
    row, gidx = _host_consts(knn_inds)
    fi = np.asarray(feature_i, dtype=np.float32).reshape(NCORES, BPC, NF, C, H * W)
    # [core, b, nf, c, q] -> [core, nf, b, c, q], f16
    fi = np.ascontiguousarray(fi.transpose(0, 2, 1, 3, 4)).astype(np.float16)
    fj = np.asarray(feature_j, dtype=np.float32).reshape(NCORES, BPC, NF, C, H * W)
    # [core, b, nf, c, q] -> [core, nf, q, b, c]: 512B rows carrying both
    # batches' channels for one spatial position
    fjt = np.ascontiguousarray(fj.transpose(0, 2, 4, 1, 3)).astype(np.float16)
    gidx2 = gidx.reshape(P, NF * (NIDX // 16))
    in_maps = []
    for core in range(NCORES):
        in_maps.append(
            {
                "fi": fi[core],
                "fjt": fjt[core].reshape(NF, H * W, BPC * C),
                "crow": row,
                "gidx": gidx2,
            }
        )
    return in_maps


def kernel(feature_i, feature_j, mask, optical_flow, knn_inds):
    from concourse import bass_utils

    nc = _get_bass()
    in_maps = _prepare_in_maps(feature_i, feature_j, knn_inds)

    res = bass_utils.run_bass_kernel_spmd(nc, in_maps, core_ids=list(range(NCORES)))
    out = np.stack([res.results[c]["out"] for c in range(NCORES)], axis=0)
    out = out.reshape(NCORES, NF, BPC, K, G, G).transpose(0, 2, 1, 3, 4, 5)
    return np.ascontiguousarray(out.reshape(B, NF, K, G, G)).astype(np.float32)
